# revision 1
# baseline (speedup 1.0000x reference)
"""LIF neuron scan kernel for Trainium2 (Bass/Tile), SPMD over 8 NeuronCores.

Problem: x [T=8, B=64, C=128, H=32, W=32] f32.
    mem = tau*mem + x_t; spike = (mem - 1 > 0); mem *= (1 - spike)
Returns spikes, same shape/dtype as x.

Sharding: data-parallel on B (8 per core), no cross-core communication.
Per-core layout: partition dim = C (128), free dim = H*W (1024), one tile
per (t, b). Per step the LIF update is 3 DVE ops:
    u   = (mem * tau) + x_t          scalar_tensor_tensor, 1x mode
    s   = (u > 1)                    tensor_scalar is_gt, 2x mode
    mem = (u <= 1) * u               scalar_tensor_tensor, 1x mode
with t=0 specialized to u = x_0 (mem0 == 0) and the mem update skipped on
the last step. Input DMAs ride the sync (SP) HWDGE ring, output DMAs the
scalar (ACT) ring so the two streams don't share a FIFO.
"""

import numpy as np

import concourse.bacc as bacc
import concourse.mybir as mybir
import concourse.tile as tile
from concourse.bass_utils import run_bass_kernel_spmd

T = 8
B = 64
C = 128
H = 32
W = 32
HW = H * W
NCORES = 8
BS = B // NCORES  # batch entries per core
TAU = 0.25
THRESH = 1.0

X_BUFS = 8
S_BUFS = 8
U_BUFS = 3
M_BUFS = 2

_nc_cache = None


def _build():
    global _nc_cache
    if _nc_cache is not None:
        return _nc_cache

    nc = bacc.Bacc("TRN2", target_bir_lowering=False, debug=False, num_devices=NCORES)
    f32 = mybir.dt.float32
    OP = mybir.AluOpType

    x = nc.dram_tensor("x", [T, BS, C, HW], f32, kind="ExternalInput").ap()
    s = nc.dram_tensor("spikes", [T, BS, C, HW], f32, kind="ExternalOutput").ap()

    with tile.TileContext(nc) as tc:
        with (
            tc.tile_pool(name="xp", bufs=X_BUFS) as xp,
            tc.tile_pool(name="up", bufs=U_BUFS) as up,
            tc.tile_pool(name="sp", bufs=S_BUFS) as sp,
            tc.tile_pool(name="mp", bufs=M_BUFS) as mp,
        ):
            for b in range(BS):
                mem = mp.tile([C, HW], f32, tag="mem")
                for t in range(T):
                    xt = xp.tile([C, HW], f32, tag="xt")
                    nc.sync.dma_start(xt[:], x[t, b])
                    if t == 0:
                        u = xt  # mem0 == 0 -> u = tau*0 + x_0 = x_0
                    else:
                        u = up.tile([C, HW], f32, tag="u")
                        nc.vector.scalar_tensor_tensor(
                            u[:], mem[:], TAU, xt[:], OP.mult, OP.add
                        )
                    st = sp.tile([C, HW], f32, tag="st")
                    nc.vector.tensor_scalar(st[:], u[:], THRESH, None, OP.is_gt)
                    if t < T - 1:
                        nc.vector.scalar_tensor_tensor(
                            mem[:], u[:], THRESH, u[:], OP.is_le, OP.mult
                        )
                    nc.scalar.dma_start(s[t, b], st[:])

    nc.compile()
    _nc_cache = nc
    return nc


def kernel(x: np.ndarray) -> np.ndarray:
    assert x.shape == (T, B, C, H, W), x.shape
    nc = _build()
    in_maps = [
        {"x": np.ascontiguousarray(x[:, i * BS : (i + 1) * BS]).reshape(T, BS, C, HW)}
        for i in range(NCORES)
    ]
    res = run_bass_kernel_spmd(nc, in_maps, core_ids=list(range(NCORES)))
    out = np.concatenate(
        [r["spikes"].reshape(T, BS, C, H, W) for r in res.results], axis=1
    )
    return out.astype(np.float32, copy=False)


# revision 4
# speedup vs baseline: 1.1548x; 1.1548x over previous
"""LIF neuron scan kernel for Trainium2 (Bass/Tile), SPMD over 8 NeuronCores.

Problem: x [T=8, B=64, C=128, H=32, W=32] f32.
    mem = tau*mem + x_t; spike = (mem - 1 > 0); mem *= (1 - spike)
Returns spikes, same shape/dtype as x.

Sharding: data-parallel on B (8 per core), no cross-core communication.
Per-core layout: partition dim = C (128), free dim = H*W (1024), one tile
per (t, b). Per step the LIF update is 3 DVE ops:
    u   = (mem * tau) + x_t          scalar_tensor_tensor, 1x mode
    s   = (u > 1)                    tensor_scalar is_gt, 2x mode
    mem = (u <= 1) * u               scalar_tensor_tensor, 1x mode
with t=0 specialized to u = x_0 (mem0 == 0) and the mem update skipped on
the last step. Input DMAs ride the sync (SP) HWDGE ring, output DMAs the
scalar (ACT) ring so the two streams don't share a FIFO.
"""

import numpy as np

import concourse.bacc as bacc
import concourse.mybir as mybir
import concourse.tile as tile
from concourse.bass_utils import run_bass_kernel_spmd

T = 8
B = 64
C = 128
H = 32
W = 32
HW = H * W
NCORES = 8
BS = B // NCORES  # batch entries per core
TAU = 0.25
THRESH = 1.0

X_BUFS = 14
S_BUFS = 12
U_BUFS = 8
M_BUFS = 8

_nc_cache = None


def _build():
    global _nc_cache
    if _nc_cache is not None:
        return _nc_cache

    nc = bacc.Bacc("TRN2", target_bir_lowering=False, debug=False, num_devices=NCORES)
    f32 = mybir.dt.float32
    OP = mybir.AluOpType

    x = nc.dram_tensor("x", [T, BS, C, HW], f32, kind="ExternalInput").ap()
    s = nc.dram_tensor("spikes", [T, BS, C, HW], f32, kind="ExternalOutput").ap()

    with tile.TileContext(nc) as tc:
        with (
            tc.tile_pool(name="xp", bufs=X_BUFS) as xp,
            tc.tile_pool(name="up", bufs=U_BUFS) as up,
            tc.tile_pool(name="sp", bufs=S_BUFS) as sp,
            tc.tile_pool(name="mp", bufs=M_BUFS) as mp,
        ):
            # t-outer / b-inner: all 8 per-b scan chains advance together, so
            # the kernel tail is 8 independent (is_gt + store) pairs instead
            # of one serial chain. mem tiles (one per b) live the whole kernel.
            mems = [
                mp.tile([C, HW], f32, tag="mem", name=f"mem{b}") for b in range(BS)
            ]
            for t in range(T):
                for b in range(BS):
                    mem = mems[b]
                    xt = xp.tile([C, HW], f32, tag="xt")
                    nc.sync.dma_start(xt[:], x[t, b])
                    if t == 0:
                        u = xt  # mem0 == 0 -> u = tau*0 + x_0 = x_0
                    else:
                        u = up.tile([C, HW], f32, tag="u")
                        nc.vector.scalar_tensor_tensor(
                            u[:], mem[:], TAU, xt[:], OP.mult, OP.add
                        )
                    st = sp.tile([C, HW], f32, tag="st")
                    nc.vector.tensor_scalar(st[:], u[:], THRESH, None, OP.is_gt)
                    if t < T - 1:
                        nc.vector.scalar_tensor_tensor(
                            mem[:], u[:], THRESH, u[:], OP.is_le, OP.mult
                        )
                    nc.scalar.dma_start(s[t, b], st[:])

    nc.compile()
    _nc_cache = nc
    return nc


def kernel(x: np.ndarray) -> np.ndarray:
    assert x.shape == (T, B, C, H, W), x.shape
    nc = _build()
    in_maps = [
        {"x": np.ascontiguousarray(x[:, i * BS : (i + 1) * BS]).reshape(T, BS, C, HW)}
        for i in range(NCORES)
    ]
    res = run_bass_kernel_spmd(nc, in_maps, core_ids=list(range(NCORES)))
    out = np.concatenate(
        [r["spikes"].reshape(T, BS, C, H, W) for r in res.results], axis=1
    )
    return out.astype(np.float32, copy=False)


# revision 6
# speedup vs baseline: 1.1913x; 1.0316x over previous
"""LIF neuron scan kernel for Trainium2 (Bass/Tile), SPMD over 8 NeuronCores.

Problem: x [T=8, B=64, C=128, H=32, W=32] f32.
    mem = tau*mem + x_t; spike = (mem - 1 > 0); mem *= (1 - spike)
Returns spikes, same shape/dtype as x.

Sharding: data-parallel on B (8 per core), no cross-core communication.
Per-core layout: partition dim = C (128), free dim = H*W (1024), one tile
per (t, b). Per step the LIF update is 3 DVE ops:
    u   = (mem * tau) + x_t          scalar_tensor_tensor, 1x mode
    s   = (u > 1)                    tensor_scalar is_gt, 2x mode
    mem = (u <= 1) * u               scalar_tensor_tensor, 1x mode
with t=0 specialized to u = x_0 (mem0 == 0) and the mem update skipped on
the last step. Input DMAs ride the sync (SP) HWDGE ring, output DMAs the
scalar (ACT) ring so the two streams don't share a FIFO.
"""

import numpy as np

import concourse.bacc as bacc
import concourse.mybir as mybir
import concourse.tile as tile
from concourse.bass_utils import run_bass_kernel_spmd

T = 8
B = 64
C = 128
H = 32
W = 32
HW = H * W
NCORES = 8
BS = B // NCORES  # batch entries per core
TAU = 0.25
THRESH = 1.0

X_BUFS = 14
S_BUFS = 12
U_BUFS = 8
M_BUFS = 8

_nc_cache = None


def _build():
    global _nc_cache
    if _nc_cache is not None:
        return _nc_cache

    nc = bacc.Bacc("TRN2", target_bir_lowering=False, debug=False, num_devices=NCORES)
    f32 = mybir.dt.float32
    OP = mybir.AluOpType

    x = nc.dram_tensor("x", [T, BS, C, HW], f32, kind="ExternalInput").ap()
    s = nc.dram_tensor("spikes", [T, BS, C, HW], f32, kind="ExternalOutput").ap()

    with tile.TileContext(nc) as tc:
        with (
            tc.tile_pool(name="xp", bufs=X_BUFS) as xp,
            tc.tile_pool(name="up", bufs=U_BUFS) as up,
            tc.tile_pool(name="sp", bufs=S_BUFS) as sp,
            tc.tile_pool(name="mp", bufs=M_BUFS) as mp,
        ):
            # t-outer / b-inner: all 8 per-b scan chains advance together, so
            # the kernel tail is 8 independent (is_gt + store) pairs instead
            # of one serial chain. mem tiles (one per b) live the whole kernel.
            mems = [
                mp.tile([C, HW], f32, tag="mem", name=f"mem{b}") for b in range(BS)
            ]
            for t in range(T):
                for b in range(BS):
                    mem = mems[b]
                    xt = xp.tile([C, HW], f32, tag="xt")
                    nc.sync.dma_start(xt[:], x[t, b])
                    if t == 0:
                        u = xt  # mem0 == 0 -> u = tau*0 + x_0 = x_0
                    else:
                        u = up.tile([C, HW], f32, tag="u")
                        nc.vector.scalar_tensor_tensor(
                            u[:], mem[:], TAU, xt[:], OP.mult, OP.add
                        )
                    st = sp.tile([C, HW], f32, tag="st")
                    # Spike compare: split between DVE (is_gt, 2x mode) and
                    # ACT (Relu(Sign(u-1)), exact in fp32) to balance engines.
                    # Last t-row stays on DVE so the kernel tail is short;
                    # t=0 goes to ACT while DVE does the 8 mem-reset ops.
                    on_act = t == 0 or (0 < t < T - 1 and b % 2 == 1)
                    if on_act:
                        # st = Sign(1 - u); st = Relu(-st)  ==  (u > 1)
                        # (bias +1.0 has a pre-registered const AP; -1.0 does
                        # not. fl(1-u) = -fl(u-1) exactly, so this is exact.)
                        nc.scalar.activation(
                            st[:],
                            u[:],
                            mybir.ActivationFunctionType.Sign,
                            bias=THRESH,
                            scale=-1.0,
                        )
                        nc.scalar.activation(
                            st[:],
                            st[:],
                            mybir.ActivationFunctionType.Relu,
                            bias=0.0,
                            scale=-1.0,
                        )
                    else:
                        nc.vector.tensor_scalar(st[:], u[:], THRESH, None, OP.is_gt)
                    if t < T - 1:
                        nc.vector.scalar_tensor_tensor(
                            mem[:], u[:], THRESH, u[:], OP.is_le, OP.mult
                        )
                    nc.scalar.dma_start(s[t, b], st[:])

    nc.compile()
    _nc_cache = nc
    return nc


def kernel(x: np.ndarray) -> np.ndarray:
    assert x.shape == (T, B, C, H, W), x.shape
    nc = _build()
    in_maps = [
        {"x": np.ascontiguousarray(x[:, i * BS : (i + 1) * BS]).reshape(T, BS, C, HW)}
        for i in range(NCORES)
    ]
    res = run_bass_kernel_spmd(nc, in_maps, core_ids=list(range(NCORES)))
    out = np.concatenate(
        [r["spikes"].reshape(T, BS, C, H, W) for r in res.results], axis=1
    )
    return out.astype(np.float32, copy=False)
